# revision 12
# baseline (speedup 1.0000x reference)
"""TRN2 Bass kernel: 2-layer bidirectional LSTM encoder (nn_BiLstmCellEncoder).

Full-input contract: kernel(**inputs) takes the unsharded inputs of
reference.setup_inputs() and returns the full [128, 200, 1024] fp32 output.

Sharding: the forward chain (layer0->layer1 fwd) and backward chain are
completely independent, so work is split direction (2) x batch quarters (4)
across the 8 NeuronCores. Every core runs the SAME SPMD program: a 2-layer
unidirectional LSTM over 32 sequences; the backward direction is realized by
feeding time-reversed inputs/mask and reversing the output on the host.

The two layer recurrences are interleaved step-by-step with a 2-block lag
(layer 1 consumes layer 0's masked output through per-block DRAM handoff
tiles). The PE alternates L0-step and L1-step matmul groups, so each layer's
serial gate chain (PE->DVE->ACT->DVE->ACT->DVE) hides inside the other
layer's matmul stream. Input projections (gx = Wih @ x + b) are produced one
block ahead, interleaved 2 gate-chunks per step. Matmul operands are bf16
(fp32 PSUM accumulation, fp32 cell state).
"""
import sys
sys.path.insert(0, '/opt/trn_rl_repo')

import numpy as np
import ml_dtypes

import concourse.bass as bass
import concourse.mybir as mybir
from concourse import bacc
import concourse.tile as tile
from concourse import bass_utils

F32 = mybir.dt.float32
BF16 = mybir.dt.bfloat16
AF = mybir.ActivationFunctionType
OP = mybir.AluOpType

B, S, D, H = 128, 200, 512, 512
NG = 4 * H
KC = 4    # 128-row chunks over H/D (contraction)
MC = 16   # 128-row chunks over the 4H gate rows
BC = 32   # batch per core
TB = 8    # time-steps per pipeline block (must divide S)
LAG = 2   # blocks of lag between layer 0 and layer 1
NCORES = 8

TRACE = False
LAST_RESULTS = None


def _build():
    assert S % TB == 0
    nblocks = S // TB
    nc = bacc.Bacc("TRN2", target_bir_lowering=False, debug=False,
                   num_devices=NCORES)

    xT = nc.dram_tensor("xT", [KC, 128, S, BC], BF16, kind="ExternalInput")
    wih = [nc.dram_tensor(f"wih{l}T", [KC, 128, NG], BF16, kind="ExternalInput")
           for l in range(2)]
    whh = [nc.dram_tensor(f"whh{l}T", [KC, 128, NG], BF16, kind="ExternalInput")
           for l in range(2)]
    bias = [nc.dram_tensor(f"bias{l}T", [128, MC], F32, kind="ExternalInput")
            for l in range(2)]
    maskrep = nc.dram_tensor("maskrep", [128, S, BC], BF16, kind="ExternalInput")
    out = nc.dram_tensor("out", [KC, 128, S, BC], BF16, kind="ExternalOutput")

    with tile.TileContext(nc) as tc:
        with tc.tile_pool(name="const", bufs=1) as cpool, \
             tc.tile_pool(name="state", bufs=1) as spool, \
             tc.tile_pool(name="stage", bufs=4) as stpool, \
             tc.tile_pool(name="gx", bufs=4) as gxpool, \
             tc.tile_pool(name="win", bufs=2) as wpool, \
             tc.tile_pool(name="act", bufs=4) as apool, \
             tc.tile_pool(name="dram", bufs=1, space="DRAM") as dpool, \
             tc.tile_pool(name="psg", bufs=5, space="PSUM") as psg_pool, \
             tc.tile_pool(name="psp", bufs=3, space="PSUM") as psp_pool:

            wih_sb = [cpool.tile([128, KC, NG], BF16, tag=f"wih{l}",
                                 name=f"wih{l}sb") for l in range(2)]
            whh_sb = [cpool.tile([128, KC, NG], BF16, tag=f"whh{l}",
                                 name=f"whh{l}sb") for l in range(2)]
            bias_sb = [cpool.tile([128, MC], F32, tag=f"bias{l}",
                                  name=f"bias{l}sb") for l in range(2)]
            mask_sb = cpool.tile([128, S, BC], BF16, tag="mask")
            for l in range(2):
                nc.sync.dma_start(wih_sb[l][:],
                                  wih[l].ap().rearrange("k p c -> p k c"))
                nc.sync.dma_start(whh_sb[l][:],
                                  whh[l].ap().rearrange("k p c -> p k c"))
                nc.sync.dma_start(bias_sb[l][:], bias[l].ap())
            nc.sync.dma_start(mask_sb[:], maskrep.ap())

            h_sbs = [spool.tile([128, KC, BC], BF16, tag=f"h{l}",
                                name=f"h{l}") for l in range(2)]
            c_sbs = [spool.tile([128, KC, BC], F32, tag=f"c{l}",
                                name=f"c{l}") for l in range(2)]
            for l in range(2):
                nc.vector.memset(h_sbs[l][:], 0.0)
                nc.vector.memset(c_sbs[l][:], 0.0)

            # per-block handoff tiles: L1's reads depend only on the matching
            # L0 block (Tile tracks DRAM deps per tile)
            h0m_blk = [dpool.tile([KC, 128, TB, BC], BF16, tag=f"h0m{k}",
                                  name=f"h0m{k}") for k in range(nblocks)]

            def stage_block(l, k):
                st = stpool.tile([128, KC, TB, BC], BF16, tag="stage",
                                 name="st")
                if l == 0:
                    src = xT.ap()[:, :, k * TB:(k + 1) * TB, :]
                else:
                    src = h0m_blk[k][:]
                nc.sync.dma_start(st[:], src.rearrange("k p t b -> p k t b"))
                return st

            def produce(l, st, gx, m):
                pp = psp_pool.tile([128, TB, BC], F32, tag="psp", name="pp")
                for kc in range(KC):
                    nc.tensor.matmul(
                        pp[:],
                        wih_sb[l][:, kc, m * 128:(m + 1) * 128],
                        st[:, kc, :, :],
                        start=(kc == 0), stop=(kc == KC - 1))
                nc.scalar.activation(gx[:, :, m, :], pp[:], AF.Identity,
                                     bias=bias_sb[l][:, m:m + 1])

            ctx = [dict(), dict()]

            def block_pre(l, kb):
                c = ctx[l]
                if kb == 0:
                    c['st_cur'] = stage_block(l, 0)
                    c['gx_cur'] = gxpool.tile([128, TB, MC, BC], F32,
                                              tag="gx", name="gx0")
                    for m in range(MC):
                        produce(l, c['st_cur'], c['gx_cur'], m)
                if kb + 1 < nblocks:
                    c['st_next'] = stage_block(l, kb + 1)
                    c['gx_next'] = gxpool.tile([128, TB, MC, BC], F32,
                                               tag="gx", name="gxn")
                c['win'] = wpool.tile([128, KC, TB, BC], BF16,
                                      tag=f"win{l}", name="win")

            def emit_step(l, kb, j):
                c = ctx[l]
                h_sb, c_sb = h_sbs[l], c_sbs[l]
                gx_cur, win = c['gx_cur'], c['win']
                t = kb * TB + j
                # gate-row chunks are host-reordered to [i(0-3) f(4-7)
                # o(8-11) g(12-15)], so psum slot m == weight chunk m
                pg = psg_pool.tile([128, 16, BC], F32, tag="psg",
                                   name="pg")
                for m in range(MC):
                    for kc in range(KC):
                        nc.tensor.matmul(
                            pg[:, m, :],
                            whh_sb[l][:, kc, m * 128:(m + 1) * 128],
                            h_sb[:, kc, :],
                            start=(kc == 0), stop=(kc == KC - 1))
                if kb + 1 < nblocks:
                    for mm in range(j * MC // TB, (j + 1) * MC // TB):
                        produce(l, c['st_next'], c['gx_next'], mm)
                gifo = apool.tile([128, 12, BC], F32, tag="gifo", name="gifo")
                nc.vector.tensor_tensor(gifo[:], pg[:, 0:12, :],
                                        gx_cur[:, j, 0:12, :], OP.add)
                gg = apool.tile([128, 4, BC], F32, tag="gg", name="gg")
                nc.vector.tensor_tensor(gg[:], pg[:, 12:16, :],
                                        gx_cur[:, j, 12:16, :], OP.add)
                sifo = apool.tile([128, 12, BC], F32, tag="sifo",
                                  name="sifo")
                t_g = apool.tile([128, 4, BC], F32, tag="t_g", name="t_g")
                nc.scalar.activation(sifo[:], gifo[:], AF.Sigmoid)
                nc.scalar.activation(t_g[:], gg[:], AF.Tanh)
                tmp = apool.tile([128, 4, BC], F32, tag="tmp", name="tmp")
                nc.vector.tensor_tensor(tmp[:], sifo[:, 0:4, :], t_g[:],
                                        OP.mult)
                nc.vector.tensor_tensor(c_sb[:], c_sb[:], sifo[:, 4:8, :],
                                        OP.mult)
                nc.vector.tensor_tensor(c_sb[:], c_sb[:], tmp[:], OP.add)
                tc_t = apool.tile([128, 4, BC], F32, tag="tc", name="tc_t")
                nc.scalar.activation(tc_t[:], c_sb[:], AF.Tanh)
                nc.vector.tensor_tensor(h_sb[:], sifo[:, 8:12, :], tc_t[:],
                                        OP.mult)
                nc.vector.tensor_tensor(
                    win[:, :, j, :], h_sb[:],
                    mask_sb[:, t, None, :].to_broadcast([128, KC, BC]),
                    OP.mult)

            def block_post(l, kb):
                c = ctx[l]
                if l == 0:
                    dst = h0m_blk[kb][:]
                else:
                    dst = out.ap()[:, :, kb * TB:(kb + 1) * TB, :]
                nc.sync.dma_start(dst.rearrange("k p t b -> p k t b"),
                                  c['win'][:])
                if kb + 1 < nblocks:
                    c['st_cur'] = c['st_next']
                    c['gx_cur'] = c['gx_next']

            for k in range(nblocks + LAG):
                l0 = k < nblocks
                l1 = k >= LAG
                if l0:
                    block_pre(0, k)
                if l1:
                    block_pre(1, k - LAG)
                for j in range(TB):
                    if l0:
                        emit_step(0, k, j)
                    if l1:
                        emit_step(1, k - LAG, j)
                if l0:
                    block_post(0, k)
                if l1:
                    block_post(1, k - LAG)
    nc.compile()
    return nc


_NC = None


def _get_nc():
    global _NC
    if _NC is None:
        _NC = _build()
    return _NC


def _prep_in_maps(x, lens, Wih_f, Whh_f, bih_f, bhh_f, Wih_b, Whh_b,
                  bih_b, bhh_b):
    bf = ml_dtypes.bfloat16
    x = np.asarray(x, dtype=np.float32)
    lens_np = np.asarray(lens).astype(np.int64)
    valid_full = (np.arange(S)[None, :] < lens_np[:, None]).astype(np.float32)

    # reorder gate rows from PyTorch [i,f,g,o] to kernel [i,f,o,g] so the
    # i/f/o sigmoid rows are contiguous in the psum bank
    perm = np.concatenate([np.arange(0, H), np.arange(H, 2 * H),
                           np.arange(3 * H, 4 * H), np.arange(2 * H, 3 * H)])
    Ws = {0: (np.asarray(Wih_f)[:, perm], np.asarray(Whh_f)[:, perm],
              np.asarray(bih_f)[:, perm], np.asarray(bhh_f)[:, perm]),
          1: (np.asarray(Wih_b)[:, perm], np.asarray(Whh_b)[:, perm],
              np.asarray(bih_b)[:, perm], np.asarray(bhh_b)[:, perm])}

    in_maps = []
    for c in range(NCORES):
        dirn, q = c // 4, c % 4
        bsl = slice(q * BC, (q + 1) * BC)
        xs = x[bsl]
        valid = valid_full[bsl]
        if dirn == 1:
            xs = xs[:, ::-1]
            valid = valid[:, ::-1]
        Wihs, Whhs, bihs, bhhs = Ws[dirn]
        m = {
            "xT": np.ascontiguousarray(xs.transpose(2, 1, 0)).reshape(
                KC, 128, S, BC).astype(bf),
            "maskrep": np.broadcast_to(valid.T[None], (128, S, BC)).astype(bf)
                .copy(),
        }
        for l in range(2):
            m[f"wih{l}T"] = np.ascontiguousarray(Wihs[l].T).reshape(
                KC, 128, NG).astype(bf)
            m[f"whh{l}T"] = np.ascontiguousarray(Whhs[l].T).reshape(
                KC, 128, NG).astype(bf)
            m[f"bias{l}T"] = np.ascontiguousarray(
                (bihs[l] + bhhs[l]).astype(np.float32).reshape(MC, 128).T)
        in_maps.append(m)
    return in_maps


def _assemble(results):
    outp = np.empty((B, S, 2 * H), dtype=np.float32)
    for c in range(NCORES):
        dirn, q = c // 4, c % 4
        arr = results[c]["out"].astype(np.float32).reshape(H, S, BC) \
            .transpose(2, 1, 0)
        if dirn == 1:
            arr = arr[:, ::-1, :]
        outp[q * BC:(q + 1) * BC, :, dirn * H:(dirn + 1) * H] = arr
    return outp


def kernel(x, lens, Wih_f, Whh_f, bih_f, bhh_f, Wih_b, Whh_b, bih_b, bhh_b):
    global LAST_RESULTS
    in_maps = _prep_in_maps(x, lens, Wih_f, Whh_f, bih_f, bhh_f,
                            Wih_b, Whh_b, bih_b, bhh_b)
    nc = _get_nc()
    res = bass_utils.run_bass_kernel_spmd(nc, in_maps, list(range(NCORES)),
                                          trace=TRACE)
    LAST_RESULTS = res
    return _assemble(res.results)



# revision 14
# speedup vs baseline: 1.0591x; 1.0591x over previous
"""TRN2 Bass kernel: 2-layer bidirectional LSTM encoder (nn_BiLstmCellEncoder).

Full-input contract: kernel(**inputs) takes the unsharded inputs of
reference.setup_inputs() and returns the full [128, 200, 1024] fp32 output.

Sharding: the forward chain (layer0->layer1 fwd) and backward chain are
completely independent, so work is split direction (2) x batch quarters (4)
across the 8 NeuronCores. Every core runs the SAME SPMD program: a 2-layer
unidirectional LSTM over 32 sequences; the backward direction is realized by
feeding time-reversed inputs/mask and reversing the output on the host.

The two layer recurrences are interleaved step-by-step with a 2-block lag
(layer 1 consumes layer 0's masked output through per-block DRAM handoff
tiles). The PE alternates L0-step and L1-step matmul groups, so each layer's
serial gate chain (PE->DVE->ACT->DVE->ACT->DVE) hides inside the other
layer's matmul stream. Input projections (gx = Wih @ x + b) are produced one
block ahead, interleaved 2 gate-chunks per step. Matmul operands are bf16
(fp32 PSUM accumulation, fp32 cell state).
"""
import sys
sys.path.insert(0, '/opt/trn_rl_repo')

import numpy as np
import ml_dtypes

import concourse.bass as bass
import concourse.mybir as mybir
from concourse import bacc
import concourse.tile as tile
from concourse import bass_utils

F32 = mybir.dt.float32
BF16 = mybir.dt.bfloat16
AF = mybir.ActivationFunctionType
OP = mybir.AluOpType

B, S, D, H = 128, 200, 512, 512
NG = 4 * H
KC = 4    # 128-row chunks over H/D (contraction)
MC = 16   # 128-row chunks over the 4H gate rows
BC = 32   # batch per core
TB = 8    # time-steps per pipeline block (must divide S)
LAG = 2   # blocks of lag between layer 0 and layer 1
NCORES = 8

TRACE = False
LAST_RESULTS = None


def _build():
    assert S % TB == 0
    nblocks = S // TB
    nc = bacc.Bacc("TRN2", target_bir_lowering=False, debug=False,
                   num_devices=NCORES)

    xT = nc.dram_tensor("xT", [KC, 128, S, BC], BF16, kind="ExternalInput")
    wih = [nc.dram_tensor(f"wih{l}T", [KC, 128, NG], BF16, kind="ExternalInput")
           for l in range(2)]
    whh = [nc.dram_tensor(f"whh{l}T", [KC, 128, NG], BF16, kind="ExternalInput")
           for l in range(2)]
    bias = [nc.dram_tensor(f"bias{l}T", [128, MC], F32, kind="ExternalInput")
            for l in range(2)]
    maskrep = nc.dram_tensor("maskrep", [128, S, BC], BF16, kind="ExternalInput")
    out = nc.dram_tensor("out", [KC, 128, S, BC], BF16, kind="ExternalOutput")

    with tile.TileContext(nc) as tc:
        with tc.tile_pool(name="const", bufs=1) as cpool, \
             tc.tile_pool(name="state", bufs=1) as spool, \
             tc.tile_pool(name="stage", bufs=4) as stpool, \
             tc.tile_pool(name="gx", bufs=4) as gxpool, \
             tc.tile_pool(name="win", bufs=2) as wpool, \
             tc.tile_pool(name="act", bufs=4) as apool, \
             tc.tile_pool(name="dram", bufs=1, space="DRAM") as dpool, \
             tc.tile_pool(name="psg", bufs=5, space="PSUM") as psg_pool, \
             tc.tile_pool(name="psp", bufs=3, space="PSUM") as psp_pool:

            wih_sb = [cpool.tile([128, KC, NG], BF16, tag=f"wih{l}",
                                 name=f"wih{l}sb") for l in range(2)]
            whh_sb = [cpool.tile([128, KC, NG], BF16, tag=f"whh{l}",
                                 name=f"whh{l}sb") for l in range(2)]
            bias_sb = [cpool.tile([128, MC], F32, tag=f"bias{l}",
                                  name=f"bias{l}sb") for l in range(2)]
            mask_sb = cpool.tile([128, S, BC], BF16, tag="mask")
            for l in range(2):
                nc.sync.dma_start(wih_sb[l][:],
                                  wih[l].ap().rearrange("k p c -> p k c"))
                nc.sync.dma_start(whh_sb[l][:],
                                  whh[l].ap().rearrange("k p c -> p k c"))
                nc.sync.dma_start(bias_sb[l][:], bias[l].ap())
            nc.sync.dma_start(mask_sb[:], maskrep.ap())

            h_sbs = [spool.tile([128, KC, BC], BF16, tag=f"h{l}",
                                name=f"h{l}") for l in range(2)]
            c_sbs = [spool.tile([128, KC, BC], F32, tag=f"c{l}",
                                name=f"c{l}") for l in range(2)]
            for l in range(2):
                nc.vector.memset(h_sbs[l][:], 0.0)
                nc.vector.memset(c_sbs[l][:], 0.0)

            # per-block handoff tiles: L1's reads depend only on the matching
            # L0 block (Tile tracks DRAM deps per tile)
            h0m_blk = [dpool.tile([KC, 128, TB, BC], BF16, tag=f"h0m{k}",
                                  name=f"h0m{k}") for k in range(nblocks)]

            def stage_block(l, k):
                st = stpool.tile([128, KC, TB, BC], BF16, tag="stage",
                                 name="st")
                if l == 0:
                    src = xT.ap()[:, :, k * TB:(k + 1) * TB, :]
                else:
                    src = h0m_blk[k][:]
                nc.sync.dma_start(st[:], src.rearrange("k p t b -> p k t b"))
                return st

            def produce(l, st, gx, m):
                pp = psp_pool.tile([128, TB, BC], F32, tag="psp", name="pp")
                for kc in range(KC):
                    nc.tensor.matmul(
                        pp[:],
                        wih_sb[l][:, kc, m * 128:(m + 1) * 128],
                        st[:, kc, :, :],
                        start=(kc == 0), stop=(kc == KC - 1))
                nc.scalar.activation(gx[:, :, m, :], pp[:], AF.Identity,
                                     bias=bias_sb[l][:, m:m + 1])

            ctx = [dict(), dict()]

            def block_pre(l, kb):
                c = ctx[l]
                if kb == 0:
                    c['st_cur'] = stage_block(l, 0)
                    c['gx_cur'] = gxpool.tile([128, TB, MC, BC], F32,
                                              tag="gx", name="gx0")
                    for m in range(MC):
                        produce(l, c['st_cur'], c['gx_cur'], m)
                if kb + 1 < nblocks:
                    c['st_next'] = stage_block(l, kb + 1)
                    c['gx_next'] = gxpool.tile([128, TB, MC, BC], F32,
                                               tag="gx", name="gxn")
                c['win'] = wpool.tile([128, KC, TB, BC], BF16,
                                      tag=f"win{l}", name="win")

            def emit_step(l, kb, j):
                c = ctx[l]
                h_sb, c_sb = h_sbs[l], c_sbs[l]
                gx_cur, win = c['gx_cur'], c['win']
                t = kb * TB + j
                # gate-row chunks are host-reordered to [i(0-3) f(4-7)
                # o(8-11) g(12-15)], so psum slot m == weight chunk m
                pg = psg_pool.tile([128, 16, BC], F32, tag="psg",
                                   name="pg")
                for m in range(MC):
                    for kc in range(KC):
                        nc.tensor.matmul(
                            pg[:, m, :],
                            whh_sb[l][:, kc, m * 128:(m + 1) * 128],
                            h_sb[:, kc, :],
                            start=(kc == 0), stop=(kc == KC - 1))
                gifo = apool.tile([128, 12, BC], F32, tag="gifo", name="gifo")
                nc.vector.tensor_tensor(gifo[:], pg[:, 0:12, :],
                                        gx_cur[:, j, 0:12, :], OP.add)
                gg = apool.tile([128, 4, BC], F32, tag="gg", name="gg")
                nc.vector.tensor_tensor(gg[:], pg[:, 12:16, :],
                                        gx_cur[:, j, 12:16, :], OP.add)
                sifo = apool.tile([128, 12, BC], F32, tag="sifo",
                                  name="sifo")
                t_g = apool.tile([128, 4, BC], F32, tag="t_g", name="t_g")
                nc.scalar.activation(sifo[:], gifo[:], AF.Sigmoid)
                nc.scalar.activation(t_g[:], gg[:], AF.Tanh)
                tmp = apool.tile([128, 4, BC], F32, tag="tmp", name="tmp")
                nc.vector.tensor_tensor(tmp[:], sifo[:, 0:4, :], t_g[:],
                                        OP.mult)
                nc.vector.tensor_tensor(c_sb[:], c_sb[:], sifo[:, 4:8, :],
                                        OP.mult)
                nc.vector.tensor_tensor(c_sb[:], c_sb[:], tmp[:], OP.add)
                tc_t = apool.tile([128, 4, BC], F32, tag="tc", name="tc_t")
                nc.scalar.activation(tc_t[:], c_sb[:], AF.Tanh)
                nc.vector.tensor_tensor(h_sb[:], sifo[:, 8:12, :], tc_t[:],
                                        OP.mult)
                nc.vector.tensor_tensor(
                    win[:, :, j, :], h_sb[:],
                    mask_sb[:, t, None, :].to_broadcast([128, KC, BC]),
                    OP.mult)
                # emit next-block input projections AFTER the gate chain so
                # the chain's ACT ops aren't queued behind the psum->sbuf
                # produce copies in ACT program order
                if kb + 1 < nblocks:
                    for mm in range(j * MC // TB, (j + 1) * MC // TB):
                        produce(l, c['st_next'], c['gx_next'], mm)

            def block_post(l, kb):
                c = ctx[l]
                if l == 0:
                    dst = h0m_blk[kb][:]
                else:
                    dst = out.ap()[:, :, kb * TB:(kb + 1) * TB, :]
                nc.sync.dma_start(dst.rearrange("k p t b -> p k t b"),
                                  c['win'][:])
                if kb + 1 < nblocks:
                    c['st_cur'] = c['st_next']
                    c['gx_cur'] = c['gx_next']

            for k in range(nblocks + LAG):
                l0 = k < nblocks
                l1 = k >= LAG
                if l0:
                    block_pre(0, k)
                if l1:
                    block_pre(1, k - LAG)
                for j in range(TB):
                    if l0:
                        emit_step(0, k, j)
                    if l1:
                        emit_step(1, k - LAG, j)
                if l0:
                    block_post(0, k)
                if l1:
                    block_post(1, k - LAG)
    nc.compile()
    return nc


_NC = None


def _get_nc():
    global _NC
    if _NC is None:
        _NC = _build()
    return _NC


def _prep_in_maps(x, lens, Wih_f, Whh_f, bih_f, bhh_f, Wih_b, Whh_b,
                  bih_b, bhh_b):
    bf = ml_dtypes.bfloat16
    x = np.asarray(x, dtype=np.float32)
    lens_np = np.asarray(lens).astype(np.int64)
    valid_full = (np.arange(S)[None, :] < lens_np[:, None]).astype(np.float32)

    # reorder gate rows from PyTorch [i,f,g,o] to kernel [i,f,o,g] so the
    # i/f/o sigmoid rows are contiguous in the psum bank
    perm = np.concatenate([np.arange(0, H), np.arange(H, 2 * H),
                           np.arange(3 * H, 4 * H), np.arange(2 * H, 3 * H)])
    Ws = {0: (np.asarray(Wih_f)[:, perm], np.asarray(Whh_f)[:, perm],
              np.asarray(bih_f)[:, perm], np.asarray(bhh_f)[:, perm]),
          1: (np.asarray(Wih_b)[:, perm], np.asarray(Whh_b)[:, perm],
              np.asarray(bih_b)[:, perm], np.asarray(bhh_b)[:, perm])}

    in_maps = []
    for c in range(NCORES):
        dirn, q = c // 4, c % 4
        bsl = slice(q * BC, (q + 1) * BC)
        xs = x[bsl]
        valid = valid_full[bsl]
        if dirn == 1:
            xs = xs[:, ::-1]
            valid = valid[:, ::-1]
        Wihs, Whhs, bihs, bhhs = Ws[dirn]
        m = {
            "xT": np.ascontiguousarray(xs.transpose(2, 1, 0)).reshape(
                KC, 128, S, BC).astype(bf),
            "maskrep": np.broadcast_to(valid.T[None], (128, S, BC)).astype(bf)
                .copy(),
        }
        for l in range(2):
            m[f"wih{l}T"] = np.ascontiguousarray(Wihs[l].T).reshape(
                KC, 128, NG).astype(bf)
            m[f"whh{l}T"] = np.ascontiguousarray(Whhs[l].T).reshape(
                KC, 128, NG).astype(bf)
            m[f"bias{l}T"] = np.ascontiguousarray(
                (bihs[l] + bhhs[l]).astype(np.float32).reshape(MC, 128).T)
        in_maps.append(m)
    return in_maps


def _assemble(results):
    outp = np.empty((B, S, 2 * H), dtype=np.float32)
    for c in range(NCORES):
        dirn, q = c // 4, c % 4
        arr = results[c]["out"].astype(np.float32).reshape(H, S, BC) \
            .transpose(2, 1, 0)
        if dirn == 1:
            arr = arr[:, ::-1, :]
        outp[q * BC:(q + 1) * BC, :, dirn * H:(dirn + 1) * H] = arr
    return outp


def kernel(x, lens, Wih_f, Whh_f, bih_f, bhh_f, Wih_b, Whh_b, bih_b, bhh_b):
    global LAST_RESULTS
    in_maps = _prep_in_maps(x, lens, Wih_f, Whh_f, bih_f, bhh_f,
                            Wih_b, Whh_b, bih_b, bhh_b)
    nc = _get_nc()
    res = bass_utils.run_bass_kernel_spmd(nc, in_maps, list(range(NCORES)),
                                          trace=TRACE)
    LAST_RESULTS = res
    return _assemble(res.results)



# revision 18
# speedup vs baseline: 1.0623x; 1.0031x over previous
"""TRN2 Bass kernel: 2-layer bidirectional LSTM encoder (nn_BiLstmCellEncoder).

Full-input contract: kernel(**inputs) takes the unsharded inputs of
reference.setup_inputs() and returns the full [128, 200, 1024] fp32 output.

Sharding: the forward chain (layer0->layer1 fwd) and backward chain are
completely independent, so work is split direction (2) x batch quarters (4)
across the 8 NeuronCores. Every core runs the SAME SPMD program: a 2-layer
unidirectional LSTM over 32 sequences; the backward direction is realized by
feeding time-reversed inputs/mask and reversing the output on the host.

The two layer recurrences are interleaved step-by-step with a 2-block lag
(layer 1 consumes layer 0's masked output through per-block DRAM handoff
tiles). The PE alternates L0-step and L1-step matmul groups, so each layer's
serial gate chain (PE->DVE->ACT->DVE->ACT->DVE) hides inside the other
layer's matmul stream. Input projections (gx = Wih @ x + b) are produced one
block ahead, interleaved 2 gate-chunks per step. Matmul operands are bf16
(fp32 PSUM accumulation, fp32 cell state).
"""
import sys
sys.path.insert(0, '/opt/trn_rl_repo')

import numpy as np
import ml_dtypes

import concourse.bass as bass
import concourse.mybir as mybir
from concourse import bacc
import concourse.tile as tile
from concourse import bass_utils

F32 = mybir.dt.float32
BF16 = mybir.dt.bfloat16
AF = mybir.ActivationFunctionType
OP = mybir.AluOpType

B, S, D, H = 128, 200, 512, 512
NG = 4 * H
KC = 4    # 128-row chunks over H/D (contraction)
MC = 16   # 128-row chunks over the 4H gate rows
BC = 32   # batch per core
TB = 8    # time-steps per pipeline block (must divide S)
LAG = 2   # blocks of lag between layer 0 and layer 1
NCORES = 8

TRACE = False
LAST_RESULTS = None


def _build():
    assert S % TB == 0
    nblocks = S // TB
    nc = bacc.Bacc("TRN2", target_bir_lowering=False, debug=False,
                   num_devices=NCORES)

    xT = nc.dram_tensor("xT", [KC, 128, S, BC], BF16, kind="ExternalInput")
    wih = [nc.dram_tensor(f"wih{l}T", [KC, 128, NG], BF16, kind="ExternalInput")
           for l in range(2)]
    whh = [nc.dram_tensor(f"whh{l}T", [KC, 128, NG], BF16, kind="ExternalInput")
           for l in range(2)]
    bias = [nc.dram_tensor(f"bias{l}T", [128, MC], F32, kind="ExternalInput")
            for l in range(2)]
    maskrep = nc.dram_tensor("maskrep", [128, S, BC], BF16, kind="ExternalInput")
    out = nc.dram_tensor("out", [KC, 128, S, BC], BF16, kind="ExternalOutput")

    with tile.TileContext(nc) as tc:
        with tc.tile_pool(name="const", bufs=1) as cpool, \
             tc.tile_pool(name="state", bufs=1) as spool, \
             tc.tile_pool(name="stage", bufs=4) as stpool, \
             tc.tile_pool(name="gx", bufs=4) as gxpool, \
             tc.tile_pool(name="win", bufs=2) as wpool, \
             tc.tile_pool(name="act", bufs=4) as apool, \
             tc.tile_pool(name="psg", bufs=5, space="PSUM") as psg_pool, \
             tc.tile_pool(name="psp", bufs=3, space="PSUM") as psp_pool:

            wih_sb = [cpool.tile([128, KC, NG], BF16, tag=f"wih{l}",
                                 name=f"wih{l}sb") for l in range(2)]
            whh_sb = [cpool.tile([128, KC, NG], BF16, tag=f"whh{l}",
                                 name=f"whh{l}sb") for l in range(2)]
            bias_sb = [cpool.tile([128, MC], F32, tag=f"bias{l}",
                                  name=f"bias{l}sb") for l in range(2)]
            mask_sb = cpool.tile([128, S, BC], BF16, tag="mask")
            for l in range(2):
                nc.sync.dma_start(wih_sb[l][:],
                                  wih[l].ap().rearrange("k p c -> p k c"))
                nc.sync.dma_start(whh_sb[l][:],
                                  whh[l].ap().rearrange("k p c -> p k c"))
                nc.sync.dma_start(bias_sb[l][:], bias[l].ap())
            nc.sync.dma_start(mask_sb[:], maskrep.ap())

            h_sbs = [spool.tile([128, KC, BC], BF16, tag=f"h{l}",
                                name=f"h{l}") for l in range(2)]
            c_sbs = [spool.tile([128, KC, BC], F32, tag=f"c{l}",
                                name=f"c{l}") for l in range(2)]
            for l in range(2):
                nc.vector.memset(h_sbs[l][:], 0.0)
                nc.vector.memset(c_sbs[l][:], 0.0)

            # L0 -> L1 handoff stays in SBUF: L1 reads L0's masked-output
            # win tiles directly (same [128, KC, TB, BC] layout); the ring
            # (win0 tag, bufs=4) covers the LAG-block pipeline distance
            win_ring = {}

            def stage_block(l, k):
                if l == 1:
                    return win_ring[k]
                st = stpool.tile([128, KC, TB, BC], BF16, tag="stage",
                                 name="st")
                src = xT.ap()[:, :, k * TB:(k + 1) * TB, :]
                nc.sync.dma_start(st[:], src.rearrange("k p t b -> p k t b"))
                return st

            def produce(l, st, gx, m):
                pp = psp_pool.tile([128, TB, BC], F32, tag="psp", name="pp")
                for kc in range(KC):
                    nc.tensor.matmul(
                        pp[:],
                        wih_sb[l][:, kc, m * 128:(m + 1) * 128],
                        st[:, kc, :, :],
                        start=(kc == 0), stop=(kc == KC - 1))
                nc.scalar.activation(gx[:, :, m, :], pp[:], AF.Identity,
                                     bias=bias_sb[l][:, m:m + 1])

            ctx = [dict(), dict()]

            def block_pre(l, kb):
                c = ctx[l]
                if kb == 0:
                    c['st_cur'] = stage_block(l, 0)
                    c['gx_cur'] = gxpool.tile([128, TB, MC, BC], F32,
                                              tag="gx", name="gx0")
                    for m in range(MC):
                        produce(l, c['st_cur'], c['gx_cur'], m)
                if kb + 1 < nblocks:
                    c['st_next'] = stage_block(l, kb + 1)
                    c['gx_next'] = gxpool.tile([128, TB, MC, BC], F32,
                                               tag="gx", name="gxn")
                c['win'] = wpool.tile([128, KC, TB, BC], BF16,
                                      tag=f"win{l}", name="win",
                                      bufs=4 if l == 0 else 2)
                if l == 0:
                    win_ring[kb] = c['win']

            def emit_step(l, kb, j):
                c = ctx[l]
                h_sb, c_sb = h_sbs[l], c_sbs[l]
                gx_cur, win = c['gx_cur'], c['win']
                t = kb * TB + j
                # gate-row chunks are host-reordered to [i(0-3) f(4-7)
                # o(8-11) g(12-15)], so psum slot m == weight chunk m
                pg = psg_pool.tile([128, 16, BC], F32, tag="psg",
                                   name="pg")
                for m in range(MC):
                    for kc in range(KC):
                        nc.tensor.matmul(
                            pg[:, m, :],
                            whh_sb[l][:, kc, m * 128:(m + 1) * 128],
                            h_sb[:, kc, :],
                            start=(kc == 0), stop=(kc == KC - 1))
                gifo = apool.tile([128, 12, BC], F32, tag="gifo", name="gifo")
                nc.vector.tensor_tensor(gifo[:], pg[:, 0:12, :],
                                        gx_cur[:, j, 0:12, :], OP.add)
                gg = apool.tile([128, 4, BC], F32, tag="gg", name="gg")
                nc.vector.tensor_tensor(gg[:], pg[:, 12:16, :],
                                        gx_cur[:, j, 12:16, :], OP.add)
                sifo = apool.tile([128, 12, BC], F32, tag="sifo",
                                  name="sifo")
                t_g = apool.tile([128, 4, BC], F32, tag="t_g", name="t_g")
                nc.scalar.activation(sifo[:], gifo[:], AF.Sigmoid)
                nc.scalar.activation(t_g[:], gg[:], AF.Tanh)
                tmp = apool.tile([128, 4, BC], F32, tag="tmp", name="tmp")
                nc.vector.tensor_tensor(tmp[:], sifo[:, 0:4, :], t_g[:],
                                        OP.mult)
                nc.vector.tensor_tensor(c_sb[:], c_sb[:], sifo[:, 4:8, :],
                                        OP.mult)
                nc.vector.tensor_tensor(c_sb[:], c_sb[:], tmp[:], OP.add)
                tc_t = apool.tile([128, 4, BC], F32, tag="tc", name="tc_t")
                nc.scalar.activation(tc_t[:], c_sb[:], AF.Tanh)
                nc.vector.tensor_tensor(h_sb[:], sifo[:, 8:12, :], tc_t[:],
                                        OP.mult)
                nc.vector.tensor_tensor(
                    win[:, :, j, :], h_sb[:],
                    mask_sb[:, t, None, :].to_broadcast([128, KC, BC]),
                    OP.mult)
                # emit next-block input projections AFTER the gate chain so
                # the chain's ACT ops aren't queued behind the psum->sbuf
                # produce copies in ACT program order
                if kb + 1 < nblocks:
                    for mm in range(j * MC // TB, (j + 1) * MC // TB):
                        produce(l, c['st_next'], c['gx_next'], mm)

            def block_post(l, kb):
                c = ctx[l]
                if l == 1:
                    dst = out.ap()[:, :, kb * TB:(kb + 1) * TB, :]
                    nc.sync.dma_start(dst.rearrange("k p t b -> p k t b"),
                                      c['win'][:])
                if kb + 1 < nblocks:
                    c['st_cur'] = c['st_next']
                    c['gx_cur'] = c['gx_next']

            for k in range(nblocks + LAG):
                l0 = k < nblocks
                l1 = k >= LAG
                if l0:
                    block_pre(0, k)
                if l1:
                    block_pre(1, k - LAG)
                for j in range(TB):
                    if l0:
                        emit_step(0, k, j)
                    if l1:
                        emit_step(1, k - LAG, j)
                if l0:
                    block_post(0, k)
                if l1:
                    block_post(1, k - LAG)
    nc.compile()
    return nc


_NC = None


def _get_nc():
    global _NC
    if _NC is None:
        _NC = _build()
    return _NC


def _prep_in_maps(x, lens, Wih_f, Whh_f, bih_f, bhh_f, Wih_b, Whh_b,
                  bih_b, bhh_b):
    bf = ml_dtypes.bfloat16
    x = np.asarray(x, dtype=np.float32)
    lens_np = np.asarray(lens).astype(np.int64)
    valid_full = (np.arange(S)[None, :] < lens_np[:, None]).astype(np.float32)

    # reorder gate rows from PyTorch [i,f,g,o] to kernel [i,f,o,g] so the
    # i/f/o sigmoid rows are contiguous in the psum bank
    perm = np.concatenate([np.arange(0, H), np.arange(H, 2 * H),
                           np.arange(3 * H, 4 * H), np.arange(2 * H, 3 * H)])
    Ws = {0: (np.asarray(Wih_f)[:, perm], np.asarray(Whh_f)[:, perm],
              np.asarray(bih_f)[:, perm], np.asarray(bhh_f)[:, perm]),
          1: (np.asarray(Wih_b)[:, perm], np.asarray(Whh_b)[:, perm],
              np.asarray(bih_b)[:, perm], np.asarray(bhh_b)[:, perm])}

    in_maps = []
    for c in range(NCORES):
        dirn, q = c // 4, c % 4
        bsl = slice(q * BC, (q + 1) * BC)
        xs = x[bsl]
        valid = valid_full[bsl]
        if dirn == 1:
            xs = xs[:, ::-1]
            valid = valid[:, ::-1]
        Wihs, Whhs, bihs, bhhs = Ws[dirn]
        m = {
            "xT": np.ascontiguousarray(xs.transpose(2, 1, 0)).reshape(
                KC, 128, S, BC).astype(bf),
            "maskrep": np.broadcast_to(valid.T[None], (128, S, BC)).astype(bf)
                .copy(),
        }
        for l in range(2):
            m[f"wih{l}T"] = np.ascontiguousarray(Wihs[l].T).reshape(
                KC, 128, NG).astype(bf)
            m[f"whh{l}T"] = np.ascontiguousarray(Whhs[l].T).reshape(
                KC, 128, NG).astype(bf)
            m[f"bias{l}T"] = np.ascontiguousarray(
                (bihs[l] + bhhs[l]).astype(np.float32).reshape(MC, 128).T)
        in_maps.append(m)
    return in_maps


def _assemble(results):
    outp = np.empty((B, S, 2 * H), dtype=np.float32)
    for c in range(NCORES):
        dirn, q = c // 4, c % 4
        arr = results[c]["out"].astype(np.float32).reshape(H, S, BC) \
            .transpose(2, 1, 0)
        if dirn == 1:
            arr = arr[:, ::-1, :]
        outp[q * BC:(q + 1) * BC, :, dirn * H:(dirn + 1) * H] = arr
    return outp


def kernel(x, lens, Wih_f, Whh_f, bih_f, bhh_f, Wih_b, Whh_b, bih_b, bhh_b):
    global LAST_RESULTS
    in_maps = _prep_in_maps(x, lens, Wih_f, Whh_f, bih_f, bhh_f,
                            Wih_b, Whh_b, bih_b, bhh_b)
    nc = _get_nc()
    res = bass_utils.run_bass_kernel_spmd(nc, in_maps, list(range(NCORES)),
                                          trace=TRACE)
    LAST_RESULTS = res
    return _assemble(res.results)



# revision 20
# speedup vs baseline: 1.0910x; 1.0269x over previous
"""TRN2 Bass kernel: 2-layer bidirectional LSTM encoder (nn_BiLstmCellEncoder).

Full-input contract: kernel(**inputs) takes the unsharded inputs of
reference.setup_inputs() and returns the full [128, 200, 1024] fp32 output.

Sharding: the forward chain (layer0->layer1 fwd) and backward chain are
completely independent, so work is split direction (2) x batch quarters (4)
across the 8 NeuronCores. Every core runs the SAME SPMD program: a 2-layer
unidirectional LSTM over 32 sequences; the backward direction is realized by
feeding time-reversed inputs/mask and reversing the output on the host.

The two layer recurrences are interleaved step-by-step with a 2-block lag
(layer 1 consumes layer 0's masked output through per-block DRAM handoff
tiles). The PE alternates L0-step and L1-step matmul groups, so each layer's
serial gate chain (PE->DVE->ACT->DVE->ACT->DVE) hides inside the other
layer's matmul stream. Input projections (gx = Wih @ x + b) are produced one
block ahead, interleaved 2 gate-chunks per step. Matmul operands are bf16
(fp32 PSUM accumulation, fp32 cell state).
"""
import sys
sys.path.insert(0, '/opt/trn_rl_repo')

import numpy as np
import ml_dtypes

import concourse.bass as bass
import concourse.mybir as mybir
from concourse import bacc
import concourse.tile as tile
from concourse import bass_utils

F32 = mybir.dt.float32
BF16 = mybir.dt.bfloat16
AF = mybir.ActivationFunctionType
OP = mybir.AluOpType

B, S, D, H = 128, 200, 512, 512
NG = 4 * H
KC = 4    # 128-row chunks over H/D (contraction)
MC = 16   # 128-row chunks over the 4H gate rows
BC = 32   # batch per core
TB = 8    # time-steps per pipeline block (must divide S)
LAG = 2   # blocks of lag between layer 0 and layer 1
NCORES = 8

TRACE = False
LAST_RESULTS = None


def _build():
    assert S % TB == 0
    nblocks = S // TB
    nc = bacc.Bacc("TRN2", target_bir_lowering=False, debug=False,
                   num_devices=NCORES)

    xT = nc.dram_tensor("xT", [KC, 128, S, BC], BF16, kind="ExternalInput")
    wih = [nc.dram_tensor(f"wih{l}T", [KC, 128, NG], BF16, kind="ExternalInput")
           for l in range(2)]
    whh = [nc.dram_tensor(f"whh{l}T", [KC, 128, NG], BF16, kind="ExternalInput")
           for l in range(2)]
    bias = [nc.dram_tensor(f"bias{l}T", [128, MC], F32, kind="ExternalInput")
            for l in range(2)]
    maskrep = nc.dram_tensor("maskrep", [128, S, BC], BF16, kind="ExternalInput")
    out = nc.dram_tensor("out", [KC, 128, S, BC], BF16, kind="ExternalOutput")

    with tile.TileContext(nc) as tc:
        with tc.tile_pool(name="const", bufs=1) as cpool, \
             tc.tile_pool(name="state", bufs=1) as spool, \
             tc.tile_pool(name="stage", bufs=4) as stpool, \
             tc.tile_pool(name="gx", bufs=4) as gxpool, \
             tc.tile_pool(name="win", bufs=2) as wpool, \
             tc.tile_pool(name="act", bufs=4) as apool, \
             tc.tile_pool(name="psg", bufs=5, space="PSUM") as psg_pool, \
             tc.tile_pool(name="psp", bufs=3, space="PSUM") as psp_pool:

            wih_sb = [cpool.tile([128, KC, NG], BF16, tag=f"wih{l}",
                                 name=f"wih{l}sb") for l in range(2)]
            whh_sb = [cpool.tile([128, KC, NG], BF16, tag=f"whh{l}",
                                 name=f"whh{l}sb") for l in range(2)]
            bias_sb = [cpool.tile([128, MC], F32, tag=f"bias{l}",
                                  name=f"bias{l}sb") for l in range(2)]
            mask_sb = cpool.tile([128, S, BC], BF16, tag="mask")
            for l in range(2):
                nc.sync.dma_start(wih_sb[l][:],
                                  wih[l].ap().rearrange("k p c -> p k c"))
                nc.sync.dma_start(whh_sb[l][:],
                                  whh[l].ap().rearrange("k p c -> p k c"))
                nc.sync.dma_start(bias_sb[l][:], bias[l].ap())
            nc.sync.dma_start(mask_sb[:], maskrep.ap())

            h_sbs = [spool.tile([128, KC, BC], BF16, tag=f"h{l}",
                                name=f"h{l}") for l in range(2)]
            c_sbs = [spool.tile([128, KC, BC], F32, tag=f"c{l}",
                                name=f"c{l}") for l in range(2)]
            for l in range(2):
                nc.vector.memset(h_sbs[l][:], 0.0)
                nc.vector.memset(c_sbs[l][:], 0.0)

            # L0 -> L1 handoff stays in SBUF: L1 reads L0's masked-output
            # win tiles directly (same [128, KC, TB, BC] layout); the ring
            # (win0 tag, bufs=4) covers the LAG-block pipeline distance
            win_ring = {}

            def stage_block(l, k):
                if l == 1:
                    return win_ring[k]
                st = stpool.tile([128, KC, TB, BC], BF16, tag="stage",
                                 name="st")
                src = xT.ap()[:, :, k * TB:(k + 1) * TB, :]
                nc.sync.dma_start(st[:], src.rearrange("k p t b -> p k t b"))
                return st

            def produce(l, st, gx, m):
                pp = psp_pool.tile([128, TB, BC], F32, tag="psp", name="pp")
                for kc in range(KC):
                    nc.tensor.matmul(
                        pp[:],
                        wih_sb[l][:, kc, m * 128:(m + 1) * 128],
                        st[:, kc, :, :],
                        start=(kc == 0), stop=(kc == KC - 1))
                # bias-add + copy on DVE: keeps ACT free for the serial
                # gate-activation chain (ACT has no exec queue)
                nc.vector.tensor_scalar_add(gx[:, :, m, :], pp[:],
                                            bias_sb[l][:, m:m + 1])

            ctx = [dict(), dict()]

            def block_pre(l, kb):
                c = ctx[l]
                if kb == 0:
                    c['st_cur'] = stage_block(l, 0)
                    c['gx_cur'] = gxpool.tile([128, TB, MC, BC], F32,
                                              tag="gx", name="gx0")
                    for m in range(MC):
                        produce(l, c['st_cur'], c['gx_cur'], m)
                if kb + 1 < nblocks:
                    c['st_next'] = stage_block(l, kb + 1)
                    c['gx_next'] = gxpool.tile([128, TB, MC, BC], F32,
                                               tag="gx", name="gxn")
                c['win'] = wpool.tile([128, KC, TB, BC], BF16,
                                      tag=f"win{l}", name="win",
                                      bufs=4 if l == 0 else 2)
                if l == 0:
                    win_ring[kb] = c['win']

            def emit_step(l, kb, j):
                c = ctx[l]
                h_sb, c_sb = h_sbs[l], c_sbs[l]
                gx_cur, win = c['gx_cur'], c['win']
                t = kb * TB + j
                # gate-row chunks are host-reordered to [i(0-3) f(4-7)
                # o(8-11) g(12-15)], so psum slot m == weight chunk m
                pg = psg_pool.tile([128, 16, BC], F32, tag="psg",
                                   name="pg")
                for m in range(MC):
                    for kc in range(KC):
                        nc.tensor.matmul(
                            pg[:, m, :],
                            whh_sb[l][:, kc, m * 128:(m + 1) * 128],
                            h_sb[:, kc, :],
                            start=(kc == 0), stop=(kc == KC - 1))
                gifo = apool.tile([128, 12, BC], F32, tag="gifo", name="gifo")
                nc.vector.tensor_tensor(gifo[:], pg[:, 0:12, :],
                                        gx_cur[:, j, 0:12, :], OP.add)
                gg = apool.tile([128, 4, BC], F32, tag="gg", name="gg")
                nc.vector.tensor_tensor(gg[:], pg[:, 12:16, :],
                                        gx_cur[:, j, 12:16, :], OP.add)
                sifo = apool.tile([128, 12, BC], F32, tag="sifo",
                                  name="sifo")
                t_g = apool.tile([128, 4, BC], F32, tag="t_g", name="t_g")
                nc.scalar.activation(sifo[:], gifo[:], AF.Sigmoid)
                nc.scalar.activation(t_g[:], gg[:], AF.Tanh)
                tmp = apool.tile([128, 4, BC], F32, tag="tmp", name="tmp")
                nc.vector.tensor_tensor(tmp[:], sifo[:, 0:4, :], t_g[:],
                                        OP.mult)
                nc.vector.tensor_tensor(c_sb[:], c_sb[:], sifo[:, 4:8, :],
                                        OP.mult)
                nc.vector.tensor_tensor(c_sb[:], c_sb[:], tmp[:], OP.add)
                tc_t = apool.tile([128, 4, BC], F32, tag="tc", name="tc_t")
                nc.scalar.activation(tc_t[:], c_sb[:], AF.Tanh)
                nc.vector.tensor_tensor(h_sb[:], sifo[:, 8:12, :], tc_t[:],
                                        OP.mult)
                nc.vector.tensor_tensor(
                    win[:, :, j, :], h_sb[:],
                    mask_sb[:, t, None, :].to_broadcast([128, KC, BC]),
                    OP.mult)
                # emit next-block input projections AFTER the gate chain so
                # the chain's ACT ops aren't queued behind the psum->sbuf
                # produce copies in ACT program order
                if kb + 1 < nblocks:
                    for mm in range(j * MC // TB, (j + 1) * MC // TB):
                        produce(l, c['st_next'], c['gx_next'], mm)

            def block_post(l, kb):
                c = ctx[l]
                if l == 1:
                    dst = out.ap()[:, :, kb * TB:(kb + 1) * TB, :]
                    nc.sync.dma_start(dst.rearrange("k p t b -> p k t b"),
                                      c['win'][:])
                if kb + 1 < nblocks:
                    c['st_cur'] = c['st_next']
                    c['gx_cur'] = c['gx_next']

            for k in range(nblocks + LAG):
                l0 = k < nblocks
                l1 = k >= LAG
                if l0:
                    block_pre(0, k)
                if l1:
                    block_pre(1, k - LAG)
                for j in range(TB):
                    if l0:
                        emit_step(0, k, j)
                    if l1:
                        emit_step(1, k - LAG, j)
                if l0:
                    block_post(0, k)
                if l1:
                    block_post(1, k - LAG)
    nc.compile()
    return nc


_NC = None


def _get_nc():
    global _NC
    if _NC is None:
        _NC = _build()
    return _NC


def _prep_in_maps(x, lens, Wih_f, Whh_f, bih_f, bhh_f, Wih_b, Whh_b,
                  bih_b, bhh_b):
    bf = ml_dtypes.bfloat16
    x = np.asarray(x, dtype=np.float32)
    lens_np = np.asarray(lens).astype(np.int64)
    valid_full = (np.arange(S)[None, :] < lens_np[:, None]).astype(np.float32)

    # reorder gate rows from PyTorch [i,f,g,o] to kernel [i,f,o,g] so the
    # i/f/o sigmoid rows are contiguous in the psum bank
    perm = np.concatenate([np.arange(0, H), np.arange(H, 2 * H),
                           np.arange(3 * H, 4 * H), np.arange(2 * H, 3 * H)])
    Ws = {0: (np.asarray(Wih_f)[:, perm], np.asarray(Whh_f)[:, perm],
              np.asarray(bih_f)[:, perm], np.asarray(bhh_f)[:, perm]),
          1: (np.asarray(Wih_b)[:, perm], np.asarray(Whh_b)[:, perm],
              np.asarray(bih_b)[:, perm], np.asarray(bhh_b)[:, perm])}

    in_maps = []
    for c in range(NCORES):
        dirn, q = c // 4, c % 4
        bsl = slice(q * BC, (q + 1) * BC)
        xs = x[bsl]
        valid = valid_full[bsl]
        if dirn == 1:
            xs = xs[:, ::-1]
            valid = valid[:, ::-1]
        Wihs, Whhs, bihs, bhhs = Ws[dirn]
        m = {
            "xT": np.ascontiguousarray(xs.transpose(2, 1, 0)).reshape(
                KC, 128, S, BC).astype(bf),
            "maskrep": np.broadcast_to(valid.T[None], (128, S, BC)).astype(bf)
                .copy(),
        }
        for l in range(2):
            m[f"wih{l}T"] = np.ascontiguousarray(Wihs[l].T).reshape(
                KC, 128, NG).astype(bf)
            m[f"whh{l}T"] = np.ascontiguousarray(Whhs[l].T).reshape(
                KC, 128, NG).astype(bf)
            m[f"bias{l}T"] = np.ascontiguousarray(
                (bihs[l] + bhhs[l]).astype(np.float32).reshape(MC, 128).T)
        in_maps.append(m)
    return in_maps


def _assemble(results):
    outp = np.empty((B, S, 2 * H), dtype=np.float32)
    for c in range(NCORES):
        dirn, q = c // 4, c % 4
        arr = results[c]["out"].astype(np.float32).reshape(H, S, BC) \
            .transpose(2, 1, 0)
        if dirn == 1:
            arr = arr[:, ::-1, :]
        outp[q * BC:(q + 1) * BC, :, dirn * H:(dirn + 1) * H] = arr
    return outp


def kernel(x, lens, Wih_f, Whh_f, bih_f, bhh_f, Wih_b, Whh_b, bih_b, bhh_b):
    global LAST_RESULTS
    in_maps = _prep_in_maps(x, lens, Wih_f, Whh_f, bih_f, bhh_f,
                            Wih_b, Whh_b, bih_b, bhh_b)
    nc = _get_nc()
    res = bass_utils.run_bass_kernel_spmd(nc, in_maps, list(range(NCORES)),
                                          trace=TRACE)
    LAST_RESULTS = res
    return _assemble(res.results)

